# revision 34
# baseline (speedup 1.0000x reference)
"""Trainium2 Bass kernel for nn_GCN1PoolNorm: 3-layer GCN + shared BatchNorm +
global max pool + MLP head, SPMD across 8 NeuronCores.

Self-contained: takes FULL inputs, returns FULL output [N_GRAPHS, N_CLASSES].

v3 design — collective-free RDMA pipeline:
- Aggregation (unchanged from v2): per core = one 1/8 dst-shard of nodes;
  SPARSE dma_gather of h_tilde rows + one-hot fp8 matmuls accumulate
  agg[dst] per 512-dst PSUM bank; downstream W-matmul + BN-stat accum;
  BN affine+relu; *dis; transpose back to node-major.
- Table replication WITHOUT collective_compute: each core keeps a PRIVATE
  fp32 gather table per parity (tableA/tableB), laid out DELTA-MAJOR:
  rows [d*nsh, (d+1)*nsh) hold the shard of core (me XOR d). Gather
  indices are host-remapped accordingly (XOR is symmetric, so the layout
  is SPMD-uniform). Publishing a layer's activations = for each d in 1..7
  a single-dest remote_dma_broadcast (relative rdests, slot d = (0, d))
  of node-major bf16 tiles into the peer's stage slot d-1, in 4 per-unit
  pieces overlapped with the activation phase; receivers convert bf16 ->
  fp32 (ACT) and DMA into their table. Own shard (d=0) is written
  directly. BN stats go the same way ([128,2] fp32 to stage_s, parity-2
  by round; local reduce over 8 slots replaces the AllReduce).
- Sync: remote sems count arrivals (+2 per bcast); raw engine wait_ge
  with explicit nosync deps pins consumers behind the waits. Stage-reuse
  safety: per-publish ack (sem-only broadcast to all 8 incl self) gates
  the next publish's sends; sendbuf reuse gates on per-queue local sems;
  stats stage is parity-2 (causality covers 2-round separation). At exec
  end every protocol sem is rebased by -per_exec_total (subtract, not
  clear: count-conserving under cross-exec in-flight arrivals).
- Tile's single-core scheduling pass can't see remote sem updates; RDMA
  sems are pre-charged in scheduling-pass CoreSims only (build-time).
"""
import contextlib
import numpy as np
import ml_dtypes

from bass_rust import InstructionNameOrderedSet
from concourse import bacc, mybir, tile
from concourse.bass_utils import run_bass_kernel_spmd
from concourse.masks import make_identity

f32 = mybir.dt.float32
bf16 = mybir.dt.bfloat16
fp8 = mybir.dt.float8e4
i16 = mybir.dt.int16

N_CORES = 8
P = 128          # partition / block quantum
D = 64           # feature dim
HALF = 32768     # int16 gather index limit -> lo/hi table split
CC = 16          # gather chunk columns (CC*128 edge slots per chunk)
GC = 4           # columns per dma_gather call (1024 idx = SWDGE ring limit)
OG = 32          # one-hot matrices per O stream group
BN_EPS = 1e-5
UT = 49          # tiles per publish unit (single-unit publishes)

RDMA_SEM_PRECHARGE = 1 << 24
# measured on HW: cross-die broadcast slots (bit2 set) deliver to
# me^(slot^2); compensate at emission. The simulator delivers by slot
# value, so sim validation flips this off.
RMTV_XDIE_SWAP = True


@contextlib.contextmanager
def _precharge_rdma_sems_in_scheduling_pass(sem_nums):
    """Tile's scheduling pass simulates single-core and never sees remote
    sem updates -> raw wait_ge on RDMA sems would deadlock the pass.
    Pre-charge those sems in scheduling-pass CoreSims only (the real
    multi-core sim and HW enforce the true waits)."""
    from concourse import bass_interp as bi

    orig = bi.CoreSim.__init__

    def patched(self, *a, **kw):
        orig(self, *a, **kw)
        if kw.get("scheduling_pass"):
            for num in sem_nums:
                self._sim_state.update_semaphore(
                    mybir.SyncUpdate(sync_type="semaphore", id=num,
                                     update_mode="sem-add-imm",
                                     update_value=RDMA_SEM_PRECHARGE),
                    dont_satisfy_waits=False, instruction=None,
                    is_softdge_sem=None)

    bi.CoreSim.__init__ = patched
    try:
        yield
    finally:
        bi.CoreSim.__init__ = orig


def _dep(inst, *on):
    """Add scheduling (nosync) deps so Tile cannot hoist `inst` above the
    instructions in `on` (typically raw wait_ge's on the same engine)."""
    s = InstructionNameOrderedSet()
    for o in on:
        s.add(o.ins.name if hasattr(o, "ins") else o.name)
    obj = inst.ins if hasattr(inst, "ins") else inst
    obj.add_nosync_dependencies_from(s)


# ---------------------------------------------------------------- host prep

def _prep(x, edge_index, batch, n_classes):
    n_nodes = x.shape[0]
    n_graphs = int(batch.max()) + 1
    assert n_nodes % N_CORES == 0
    nsh = n_nodes // N_CORES                    # nodes per core
    ntile = (nsh + P - 1) // P                  # node tiles per core
    tsz = [min(P, nsh - t * P) for t in range(ntile)]

    src_all = np.asarray(edge_index[0], np.int64)
    dst_all = np.asarray(edge_index[1], np.int64)
    deg = np.bincount(dst_all, minlength=n_nodes).astype(np.int64)

    RW = 512
    RNG = (nsh + RW - 1) // RW                  # 512-node dst ranges per core

    # ---- per-core edge lists. src indices are remapped DELTA-MAJOR for
    # core c: owner o = src//nsh, local table row = (o ^ c)*nsh + src%nsh
    # (each core's private table holds shard of core c^d at rows d*nsh..).
    # Sorted by (dst_range, src_half, dst, src): each (range, half) group
    # is a contiguous slot run so the gather stream is a single monotone
    # sequence of chunks per layer.
    edges = []                                  # (s, dl, grp) per core
    for c in range(N_CORES):
        m = (dst_all // nsh) == c
        s_orig = src_all[m]
        s = ((s_orig // nsh) ^ c) * nsh + (s_orig % nsh)
        dl = dst_all[m] - c * nsh
        # (self loops are NOT edges here: the self contribution h_tilde[dst]
        # is added from the resident feature-major hbuf after aggregation)
        hi = (s >= HALF).astype(np.int64)
        grp = (dl // RW) * 2 + hi
        order = np.lexsort((s, dl, grp))
        s, dl, grp = s[order], dl[order], grp[order]
        edges.append((s, dl, grp))

    # SPMD = one program for all cores: pad every (range, half) group to
    # the max block count over cores; pad slots gather table row 0 of the
    # group's half and are masked by all-zero O columns.
    NGRP = RNG * 2
    gcols = np.zeros(NGRP, np.int64)            # block columns per group
    for c in range(N_CORES):
        cnt = np.bincount(edges[c][2], minlength=NGRP)
        gcols = np.maximum(gcols, (cnt + P - 1) // P)
    gbase = np.concatenate([[0], np.cumsum(gcols)])  # group -> base col
    SC = int(gbase[-1])

    keys = []
    per_edge = []
    for c in range(N_CORES):
        s, dl, grp = edges[c]
        gstart = np.searchsorted(grp, np.arange(NGRP))
        slot = gbase[grp] * P + (np.arange(s.shape[0]) - gstart[grp])
        b = slot // P
        t = dl // P
        hi = grp & 1
        # mm issue order: tile-major, each tile's lo mms then hi mms
        key = ((t * 2 + hi) << 24) | b
        keys.append(key)
        per_edge.append((s, dl, slot))
    uk = np.unique(np.concatenate(keys))
    nmm = int(uk.shape[0])
    uk_t = uk >> 25
    uk_b = uk & ((1 << 24) - 1)
    sched = [[] for _ in range(ntile)]
    for mi in range(nmm):
        sched[int(uk_t[mi])].append((mi, int(uk_b[mi])))

    # gather chunks: (base_col, ncols, is_hi), never spanning a group
    chunks = []
    for g in range(NGRP):
        for c0 in range(0, int(gcols[g]), CC):
            chunks.append((int(gbase[g]) + c0,
                           int(min(CC, gcols[g] - c0)), g & 1))

    NG = (nmm + OG - 1) // OG
    idx_reps, Ots = [], []
    for c in range(N_CORES):
        s, dl, slot = per_edge[c]
        hi_e = s >= HALF
        idx_flat = np.zeros(SC * P, np.int16)
        idx_flat[slot[~hi_e]] = s[~hi_e].astype(np.int16)
        idx_flat[slot[hi_e]] = (s[hi_e] - HALF).astype(np.int16)
        wrap = idx_flat.reshape(SC * 8, 16).T            # [16, SC*8]
        idx_reps.append(np.tile(wrap, (8, 1)))           # [128, SC*8]

        inv = np.searchsorted(uk, keys[c])
        O = np.zeros((nmm, P, P), np.uint8)
        O[inv, slot % P, dl % P] = 1
        Ot = np.zeros((NG, P, OG, P), ml_dtypes.float8_e4m3)
        Of = O.astype(ml_dtypes.float8_e4m3)
        for g in range(NG):
            k = min(OG, nmm - g * OG)
            Ot[g, :, :k, :] = Of[g * OG:g * OG + k].transpose(1, 0, 2)
        Ots.append(Ot)
        del O, Of

    # deg layouts (fp32)
    deg_pt = np.zeros((N_CORES, P, ntile), np.float32)
    deg_row = np.zeros((N_CORES, 1, nsh), np.float32)
    for c in range(N_CORES):
        dsh = deg[c * nsh:(c + 1) * nsh].astype(np.float32)
        deg_row[c, 0, :] = dsh
        for tt in range(ntile):
            deg_pt[c, :tsz[tt], tt] = dsh[tt * P:tt * P + tsz[tt]]

    # pooling segments (identical across cores required for SPMD)
    gb = np.searchsorted(batch, np.arange(n_graphs + 1))
    gpc = n_graphs // N_CORES
    loc0 = gb[:gpc + 1].copy()
    for c in range(N_CORES):
        locc = gb[c * gpc:(c + 1) * gpc + 1] - c * nsh
        assert np.array_equal(locc, loc0), "graph pattern must match across cores"
    pool_segs = []
    for tt in range(ntile):
        a, bb = tt * P, tt * P + tsz[tt]
        for g in range(gpc):
            ss, ee = max(a, int(loc0[g])), min(bb, int(loc0[g + 1]))
            if ss < ee:
                pool_segs.append((tt, ss - a, ee - a, g))

    cfg = dict(n_nodes=n_nodes, nsh=nsh, ntile=ntile, tsz=tsz,
               pool_segs=pool_segs, gpc=gpc, n_classes=n_classes,
               n_graphs=n_graphs, SC=SC, NG=NG, chunks=chunks, sched=sched)
    data = dict(idx_rep=idx_reps, Ot=Ots, deg_pt=deg_pt, deg_row=deg_row)
    return cfg, data


# ---------------------------------------------------------------- device build

def _build(cfg, reps=1):
    nsh, ntile, tsz = cfg["nsh"], cfg["ntile"], cfg["tsz"]
    ncls, gpc = cfg["n_classes"], cfg["gpc"]
    n_nodes = cfg["n_nodes"]
    SC, NG = cfg["SC"], cfg["NG"]
    nshp = ntile * P
    RW = 512

    sched0 = cfg["sched"]

    R = (nsh + RW - 1) // RW
    rsz = [min(RW, nsh - r * RW) for r in range(R)]

    # publish units: tiles [u0, u1)
    units = []
    t0 = 0
    while t0 < ntile:
        units.append((t0, min(t0 + UT, ntile)))
        t0 += UT
    NU = len(units)

    n_pubs = 1 + 2 * reps        # table0 + (L1, L2) per rep
    n_rounds = 3 * reps          # BN stat exchanges

    nc = bacc.Bacc(trn_type="TRN2", target_bir_lowering=False, debug=False,
                   num_devices=N_CORES, num_swdge_queues=4)

    x_sh = nc.dram_tensor("x_sh", [nsh, D], f32, kind="ExternalInput").ap()
    idx_in = nc.dram_tensor("idx", [P, SC * 8], i16, kind="ExternalInput").ap()
    O_in = nc.dram_tensor("O", [NG, P, OG, P], fp8, kind="ExternalInput").ap()
    deg_pt = nc.dram_tensor("deg_pt", [P, ntile], f32, kind="ExternalInput").ap()
    deg_row = nc.dram_tensor("deg_row", [1, nsh], f32, kind="ExternalInput").ap()
    Ws = [nc.dram_tensor(f"W{i}", [D, D], bf16, kind="ExternalInput").ap()
          for i in (1, 2, 3)]
    gamma = nc.dram_tensor("gamma", [D, 1], f32, kind="ExternalInput").ap()
    beta = nc.dram_tensor("beta", [D, 1], f32, kind="ExternalInput").ap()
    lin1w = nc.dram_tensor("lin1w", [D, D], bf16, kind="ExternalInput").ap()
    lin1b = nc.dram_tensor("lin1b", [D, 1], f32, kind="ExternalInput").ap()
    lin2w = nc.dram_tensor("lin2w", [D, ncls], bf16, kind="ExternalInput").ap()
    lin2b = nc.dram_tensor("lin2b", [ncls, 1], f32, kind="ExternalInput").ap()
    out = nc.dram_tensor("out", [gpc, ncls], f32, kind="ExternalOutput").ap()
    import os
    DBG = int(os.environ.get("KDBG", "0"))
    if DBG:
        dbg_stage = nc.dram_tensor("dbg_stage", [P, 7 * ntile * D], bf16,
                                   kind="ExternalOutput").ap()
        dbg_tbl = nc.dram_tensor("dbg_tbl", [n_nodes, D], f32,
                                 kind="ExternalOutput").ap()

    # private delta-major gather tables, double-buffered by publish parity
    tables = [nc.dram_tensor(f"table{pp}", [n_nodes, D], f32).ap()
              for pp in range(2)]

    # ---- protocol semaphores
    rsem_d = [nc.alloc_semaphore(f"rsem_d{u}") for u in range(NU)]
    rsem_s = nc.alloc_semaphore("rsem_s")
    ack_rsem = nc.alloc_semaphore("ack_rsem")
    lsem = [nc.alloc_semaphore(f"lsem_q{q}") for q in range(4)]
    conv_done = nc.alloc_semaphore("conv_done")
    # lsems are excluded: local send-completion updates DO fire in the
    # scheduling pass (trigger timeline), and SWDGE sems must start at 0.
    remote_sem_nums = ([s.num for s in rsem_d] +
                       [rsem_s.num, ack_rsem.num, conv_done.num])

    st = dict(pub=0, rnd=0, lcnt=[0, 0, 0, 0], qrr=0)
    # last-emitted instruction per engine: every raw wait_ge is pinned
    # behind it (nosync dep) so Tile's scheduler cannot hoist or fuse
    # waits across protocol epochs (that creates runtime deadlocks).
    anch = {}

    def _wge(builder, key, sem, val):
        w = builder.wait_ge(sem, val)
        if key in anch:
            _dep(w, anch[key])
        anch[key] = w
        return w

    with _precharge_rdma_sems_in_scheduling_pass(remote_sem_nums):
        with tile.TileContext(nc) as tc:
            with (
                tc.tile_pool(name="const", bufs=1) as cpool,
                tc.tile_pool(name="gath", bufs=6) as gpool,
                tc.tile_pool(name="gbf", bufs=6) as gbpool,
                tc.tile_pool(name="obuf", bufs=4) as opool,
                tc.tile_pool(name="work", bufs=3) as wpool,
                tc.tile_pool(name="psagg", bufs=2, space="PSUM") as ps_agg,
                tc.tile_pool(name="psz", bufs=2, space="PSUM") as ps_z,
                tc.tile_pool(name="pstr", bufs=2, space="PSUM") as ps_tr,
            ):
                # ---- residents
                idx_res = cpool.tile([P, SC * 8], i16)
                nc.sync.dma_start(out=idx_res[:], in_=idx_in[:])
                dis_pt = cpool.tile([P, ntile], f32)
                dis_rep = cpool.tile([D, nshp], f32)
                zbuf = cpool.tile([D, nshp], bf16)
                act3 = zbuf
                hbuf = cpool.tile([D, nshp], bf16)
                sums = cpool.tile([D, R], f32)
                sums2 = cpool.tile([D, R], f32)
                W_sb = [cpool.tile([D, D], bf16, tag=f"W{i}", name=f"W{i}_sb")
                        for i in range(3)]
                for i in range(3):
                    nc.sync.dma_start(out=W_sb[i][:], in_=Ws[i][:])
                gamma_sb = cpool.tile([D, 1], f32, tag="gamma")
                beta_sb = cpool.tile([D, 1], f32, tag="beta")
                nc.sync.dma_start(out=gamma_sb[:], in_=gamma[:])
                nc.sync.dma_start(out=beta_sb[:], in_=beta[:])
                l1w_sb = cpool.tile([D, D], bf16, tag="l1w")
                l1b_sb = cpool.tile([D, 1], f32, tag="l1b")
                l2w_sb = cpool.tile([D, ncls], bf16, tag="l2w")
                l2b_sb = cpool.tile([ncls, 1], f32, tag="l2b")
                nc.sync.dma_start(out=l1w_sb[:], in_=lin1w[:])
                nc.sync.dma_start(out=l1b_sb[:], in_=lin1b[:])
                nc.sync.dma_start(out=l2w_sb[:], in_=lin2w[:])
                nc.sync.dma_start(out=l2b_sb[:], in_=lin2b[:])
                ident = cpool.tile([D, D], bf16, tag="ident")
                make_identity(nc, ident[:])
                ident128 = cpool.tile([P, P], bf16, tag="ident128")
                make_identity(nc, ident128[:])
                emb = cpool.tile([D, gpc], f32, tag="emb")
                eps_sb = cpool.tile([D, 1], f32, tag="eps")
                nc.gpsimd.memset(eps_sb[:], BN_EPS)

                # RDMA staging
                sendbuf = cpool.tile([P, ntile, D], bf16, tag="sendbuf")
                stage_d = cpool.tile([P, 7, ntile, D], bf16, tag="stage_d")
                stage_s = cpool.tile([P, 2, 8, 2], f32, tag="stage_s")
                st2 = cpool.tile([P, 2, 2], f32, tag="st2")
                nc.gpsimd.memset(sendbuf[:], 0.0)
                nc.gpsimd.memset(stage_d[:], 0.0)
                nc.gpsimd.memset(stage_s[:], 0.0)
                nc.gpsimd.memset(st2[:], 0.0)

                # ---- dis
                dptf = wpool.tile([P, ntile], f32, tag="dptf")
                nc.sync.dma_start(out=dptf[:], in_=deg_pt[:])
                nc.scalar.activation(dis_pt[:], dptf[:],
                                     mybir.ActivationFunctionType.Sqrt,
                                     bias=1.0)
                nc.vector.reciprocal(dis_pt[:], dis_pt[:])
                ones1 = cpool.tile([1, D], bf16, tag="ones1")
                nc.gpsimd.memset(ones1[:], 1.0)
                for o in range(0, nsh, RW):
                    w = min(RW, nsh - o)
                    dsl = wpool.tile([1, RW], f32, tag="dsl")
                    nc.sync.dma_start(out=dsl[:, :w], in_=deg_row[:, o:o + w])
                    nc.scalar.activation(dsl[:, :w], dsl[:, :w],
                                         mybir.ActivationFunctionType.Sqrt,
                                         bias=1.0)
                    nc.vector.reciprocal(dsl[:, :w], dsl[:, :w])
                    dslb = wpool.tile([1, RW], bf16, tag="dslb")
                    nc.vector.tensor_copy(dslb[:, :w], dsl[:, :w])
                    pb = ps_z.tile([D, RW], f32, tag="zt", space="PSUM")
                    nc.tensor.matmul(pb[:, :w], lhsT=ones1[:], rhs=dslb[:, :w],
                                     start=True, stop=True)
                    nc.vector.tensor_copy(dis_rep[:, o:o + w], pb[:, :w])

                # ------------------------------------------------ publish plumbing
                def unit_rows(u):
                    a, b = units[u]
                    return a * P, min(b * P, nsh)

                def emit_unit_sends(u, gate_insts):
                    """7 single-dest relative broadcasts of
                    sendbuf[:, a:b, :] into peers' stage_d slot d-1."""
                    a, b = units[u]
                    preps = []
                    for d in range(1, 8):
                        q = 1 + (d - 1) % 3
                        dd = d if (d < 4 or not RMTV_XDIE_SWAP) else d ^ 2
                        rd = [None] * 8
                        rd[dd] = (0, dd)
                        pr = nc.gpsimd.remote_dma_broadcast(
                            out_ap=stage_d[:, d - 1, a:b, :],
                            in_ap=sendbuf[:, a:b, :],
                            remote_sem=rsem_d[u], local_sem=lsem[q],
                            rdests=rd, queue_num=q)
                        for g in gate_insts:
                            _dep(pr, g)
                        preps.append(pr)
                        st["lcnt"][q] += 16
                    for q in (1, 2, 3):
                        tg = nc.gpsimd.trigger_dma(count=None, queue_num=q)
                        anch["POOL"] = tg

                def emit_drains_and_ack(pub):
                    """Convert arrived bf16 stage slots to fp32 and DMA into
                    this parity's table; then ack the publish (stage free)."""
                    par = pub % 2
                    DUT = 16   # drain convert window (SBUF-bounded)
                    conv_insts = []
                    for u in range(NU):
                        ua, ub = units[u]
                        wt = _wge(nc.scalar, "ACT", rsem_d[u], 14 * (pub + 1))
                        for a in range(ua, ub, DUT):
                            b = min(a + DUT, ub)
                            nt = b - a
                            r0 = a * P
                            r1 = min(b * P, nsh)
                            for d in range(1, 8):
                                wrd = wpool.tile([P, DUT, D], f32, tag="wrd")
                                cv = nc.scalar.activation(
                                    wrd[:, :nt, :],
                                    stage_d[:, d - 1, a:b, :],
                                    mybir.ActivationFunctionType.Copy)
                                _dep(cv, wt)
                                conv_insts.append(cv)
                                anch["ACT"] = cv
                                base = d * nsh
                                nfull = (r1 - r0) // P
                                rem = (r1 - r0) - nfull * P
                                if nfull:
                                    dst = tables[par][base + r0:
                                                      base + r0
                                                      + nfull * P, :]
                                    nc.sync.dma_start(
                                        out=dst.rearrange(
                                            "(t p) d -> p t d", p=P),
                                        in_=wrd[:, :nfull, :])
                                if rem:
                                    dst = tables[par][base + r0 + nfull * P:
                                                      base + r1, :]
                                    nc.sync.dma_start(
                                        out=dst, in_=wrd[:rem, nfull, :])
                    # ack: all converts done -> stage slots of `pub` are free
                    si = nc.scalar.sem_inc(conv_done, 1)
                    _dep(si, *conv_insts)
                    anch["ACT"] = si
                    wc = _wge(nc.gpsimd, "POOL", conv_done, pub + 1)
                    ack = nc.gpsimd.remote_sem_update_broadcast(
                        ack_rsem, lsem[0],
                        rdests=[(0, k) for k in range(8)], queue_num=0)
                    _dep(ack, wc)
                    st["lcnt"][0] += 16
                    tg = nc.gpsimd.trigger_dma(count=None, queue_num=0)
                    anch["POOL"] = tg

                def publish_gates(pub):
                    """Gates that must precede publish `pub`'s sendbuf writes
                    (flight of pub-1 done) and its bcast preps (ack of
                    pub-1 received)."""
                    # ack of pub-1 implies ALL pub-1 flights were delivered
                    # (ack <= drains <= arrivals). nosync deps are
                    # scheduling-order only, so EACH engine that must stall
                    # needs its own runtime wait on the sem: DVE (sendbuf
                    # overwrites) and Pool (bcast preps).
                    dve_gates, pool_gates = [], []
                    if pub >= 1:
                        dve_gates.append(
                            _wge(nc.vector, "DVE", ack_rsem, 16 * pub))
                        pool_gates.append(
                            _wge(nc.gpsimd, "POOL", ack_rsem, 16 * pub))
                    return dve_gates, pool_gates

                # ---- table0 publish (pub 0): x * dis, node-major
                dve_g, pool_g = publish_gates(0)
                ui = 0
                for t in range(ntile):
                    w = tsz[t]
                    xt = wpool.tile([P, D], f32, tag="xt")
                    nc.sync.dma_start(out=xt[:w, :],
                                      in_=x_sh[t * P:t * P + w, :])
                    xb = wpool.tile([P, D], f32, tag="xb")
                    xsc = nc.scalar.activation(xb[:w, :], xt[:w, :],
                                               mybir.ActivationFunctionType.Copy,
                                               scale=dis_pt[:w, t:t + 1])
                    anch["ACT"] = xsc
                    nc.sync.dma_start(out=tables[0][t * P:t * P + w, :],
                                      in_=xb[:w, :])
                    sbc = nc.vector.tensor_copy(sendbuf[:w, t, :], xb[:w, :])
                    for g in dve_g:
                        _dep(sbc, g)
                    anch["DVE"] = sbc
                    xbb = wpool.tile([P, D], bf16, tag="xbb")
                    nc.vector.tensor_copy(xbb[:w, :], xb[:w, :])
                    pxt = ps_tr.tile([D, P], bf16, tag="trx", space="PSUM")
                    nc.tensor.transpose(pxt[:, :w], xbb[:w, :],
                                        ident128[:w, :w])
                    nc.vector.tensor_copy(hbuf[:, t * P:t * P + w],
                                          pxt[:, :w])
                    if t + 1 == units[ui][1]:
                        emit_unit_sends(ui, pool_g)
                        ui += 1
                emit_drains_and_ack(0)
                st["pub"] = 1
                if DBG:
                    dm1 = nc.sync.dma_start(
                        out=dbg_stage[:, :],
                        in_=stage_d[:].rearrange("p a b c -> p (a b c)"))
                    _dep(dm1, anch["ACT"])

                # chunk lookup: block col -> chunk index
                chunks = cfg["chunks"]
                n_ch = len(chunks)
                col2chunk = {}
                for k, (base, ncols, _) in enumerate(chunks):
                    for cc_ in range(base, base + ncols):
                        col2chunk[cc_] = k

                # ---- layers
                for rep in range(reps):
                    for li in range(3):
                        last = (li == 2)
                        Wl = W_sb[li]
                        rtab = tables[(st["pub"] - 1) % 2]

                        chunk_tiles = [None] * n_ch
                        next_issue = [0]

                        def issue_chunk(k, rtab=rtab):
                            base, ncols, ih = chunks[k]
                            g = gpool.tile([P, CC, D], f32, tag="g")
                            src_ap = rtab[HALF:n_nodes, :] if ih \
                                else rtab[0:HALF, :]
                            # <=1024 idx per call (SWDGE ring limit). Tile
                            # assigns each Pool DMA a DMASW lane round-robin
                            # mod 8, and a lane's sem is locked to one SWDGE
                            # queue — queue = global gather call counter % 4
                            # keeps lane->queue pairing consistent. (RDMA
                            # preps skip DMASW lanes so they don't shift it.)
                            for q0 in range(0, ncols, GC):
                                qw = min(GC, ncols - q0)
                                nc.gpsimd.dma_gather(
                                    out_ap=g[:, q0:q0 + qw, :], in_ap=src_ap,
                                    idxs_ap=idx_res[:, (base + q0) * 8:
                                                    (base + q0 + qw) * 8],
                                    num_idxs=qw * P, num_idxs_reg=qw * P,
                                    elem_size=D,
                                    queue_num=st["qrr"] % 4)
                                st["qrr"] += 1
                            gb = gbpool.tile([P, CC, D], bf16, tag="gb")
                            nc.scalar.activation(
                                gb[:, :ncols, :], g[:, :ncols, :],
                                mybir.ActivationFunctionType.Copy)
                            chunk_tiles[k] = (gb, base, ncols)

                        def chunk_of(bcol):
                            k = col2chunk[bcol]
                            while next_issue[0] <= k:
                                issue_chunk(next_issue[0])
                                next_issue[0] += 1
                            return chunk_tiles[k]

                        o_tiles = [None] * NG

                        def o_tile(mi):
                            g = mi // OG
                            if o_tiles[g] is None:
                                ot = opool.tile([P, OG, P], fp8, tag="O")
                                nc.sync.dma_start(out=ot[:], in_=O_in[g])
                                o_tiles[g] = ot
                            return o_tiles[g]

                        def downstream(r, ps):
                            rw = rsz[r]
                            us = wpool.tile([D, RW], f32, tag="us")
                            nc.vector.tensor_tensor(
                                out=us[:, :rw], in0=ps[:, :rw],
                                in1=hbuf[:, r * RW:r * RW + rw],
                                op=mybir.AluOpType.add)
                            u2t = wpool.tile([D, RW], bf16, tag="u2t")
                            nc.vector.tensor_tensor(
                                out=u2t[:, :rw], in0=us[:, :rw],
                                in1=dis_rep[:, r * RW:r * RW + rw],
                                op=mybir.AluOpType.mult)
                            psz = ps_z.tile([D, RW], f32, tag="zt",
                                            space="PSUM")
                            nc.tensor.matmul(psz[:, :rw], lhsT=Wl[:],
                                             rhs=u2t[:, :rw],
                                             start=True, stop=True)
                            zslice = (act3 if last else zbuf)[
                                :, r * RW:r * RW + rw]
                            nc.scalar.activation(
                                zslice, psz[:, :rw],
                                mybir.ActivationFunctionType.Copy,
                                accum_out=sums[:, r:r + 1])
                            sq = wpool.tile([D, RW], f32, tag="sq")
                            nc.scalar.activation(
                                sq[:, :rw], psz[:, :rw],
                                mybir.ActivationFunctionType.Square,
                                accum_out=sums2[:, r:r + 1])

                        for r in range(R):
                            rw = rsz[r]
                            ps = ps_agg.tile([D, RW], f32, tag="agg",
                                             space="PSUM")
                            for t in range(r * 4, min(r * 4 + 4, ntile)):
                                mms = sched0[t]
                                assert mms, "tile with no edges"
                                tw = tsz[t]
                                off = (t - r * 4) * P
                                nmms = len(mms)
                                for j, (mi, b) in enumerate(mms):
                                    gb, base, ncols = chunk_of(b)
                                    osb = o_tile(mi)
                                    nc.tensor.matmul(
                                        ps[:, off:off + tw],
                                        lhsT=gb[:, b - base, :],
                                        rhs=osb[:, mi % OG, :tw],
                                        start=(j == 0), stop=(j == nmms - 1))
                            downstream(r, ps)

                        # ---- BN stats via RDMA all-to-all
                        rnd = st["rnd"]
                        pss = rnd % 2
                        stp = st2[:, pss, :]
                        nc.vector.reduce_sum(stp[0:D, 0:1], sums[:],
                                             axis=mybir.AxisListType.X)
                        nc.vector.reduce_sum(stp[0:D, 1:2], sums2[:],
                                             axis=mybir.AxisListType.X)
                        for d in range(1, 8):
                            dd = d if (d < 4 or not RMTV_XDIE_SWAP) \
                                else d ^ 2
                            rd = [None] * 8
                            rd[dd] = (0, dd)
                            pr = nc.gpsimd.remote_dma_broadcast(
                                out_ap=stage_s[:, pss, d, :], in_ap=stp,
                                remote_sem=rsem_s, local_sem=lsem[0],
                                rdests=rd, queue_num=0)
                            st["lcnt"][0] += 16
                        tg = nc.gpsimd.trigger_dma(count=None, queue_num=0)
                        anch["POOL"] = tg
                        scp = nc.vector.tensor_copy(stage_s[:, pss, 0, :],
                                                    stp)
                        anch["DVE"] = scp

                        ws = _wge(nc.vector, "DVE", rsem_s, 14 * (rnd + 1))
                        stot = wpool.tile([P, 2], f32, tag="stot")
                        rs0 = nc.vector.reduce_sum(
                            stot[:, 0:1],
                            stage_s[:, pss, :, 0:1].rearrange(
                                "p s one -> p (s one)"),
                            axis=mybir.AxisListType.X)
                        rs1 = nc.vector.reduce_sum(
                            stot[:, 1:2],
                            stage_s[:, pss, :, 1:2].rearrange(
                                "p s one -> p (s one)"),
                            axis=mybir.AxisListType.X)
                        _dep(rs0, ws)
                        _dep(rs1, ws)
                        st["rnd"] = rnd + 1

                        mu = wpool.tile([D, 1], f32, tag="mu")
                        nc.scalar.activation(mu[:], stot[0:D, 0:1],
                                             mybir.ActivationFunctionType.Copy,
                                             scale=1.0 / n_nodes)
                        va = wpool.tile([D, 1], f32, tag="va")
                        nc.scalar.activation(va[:], stot[0:D, 1:2],
                                             mybir.ActivationFunctionType.Copy,
                                             scale=1.0 / n_nodes)
                        mu2 = wpool.tile([D, 1], f32, tag="mu2")
                        nc.vector.tensor_tensor(out=mu2[:], in0=mu[:],
                                                in1=mu[:],
                                                op=mybir.AluOpType.mult)
                        nc.vector.tensor_tensor(out=va[:], in0=va[:],
                                                in1=mu2[:],
                                                op=mybir.AluOpType.subtract)
                        nc.scalar.activation(va[:], va[:],
                                             mybir.ActivationFunctionType.Sqrt,
                                             bias=eps_sb[:])
                        nc.vector.reciprocal(va[:], va[:])
                        saff = wpool.tile([D, 1], f32, tag="saff")
                        nc.vector.tensor_tensor(out=saff[:], in0=gamma_sb[:],
                                                in1=va[:],
                                                op=mybir.AluOpType.mult)
                        tsh_ = wpool.tile([D, 1], f32, tag="tsh")
                        nc.vector.tensor_tensor(out=tsh_[:], in0=mu[:],
                                                in1=saff[:],
                                                op=mybir.AluOpType.mult)
                        tshi = nc.vector.tensor_tensor(
                            out=tsh_[:], in0=beta_sb[:], in1=tsh_[:],
                            op=mybir.AluOpType.subtract)
                        anch["DVE"] = tshi

                        # ---- activation phase (per range) + publish
                        if not last:
                            pub = st["pub"]
                            par = pub % 2
                            dve_g, pool_g = publish_gates(pub)
                            ui = 0
                        for r in range(R):
                            rw = rsz[r]
                            zsl = (act3 if last else zbuf)[
                                :, r * RW:r * RW + rw]
                            at = wpool.tile([D, RW], bf16, tag="at")
                            ati = nc.scalar.activation(
                                at[:, :rw], zsl,
                                mybir.ActivationFunctionType.Relu,
                                bias=tsh_[:], scale=saff[:])
                            anch["ACT"] = ati
                            if not last:
                                ht = hbuf[:, r * RW:r * RW + rw]
                                hti = nc.vector.tensor_tensor(
                                    out=ht, in0=at[:, :rw],
                                    in1=dis_rep[:, r * RW:r * RW + rw],
                                    op=mybir.AluOpType.mult)
                                anch["DVE"] = hti
                                for t in range(r * 4, min(r * 4 + 4, ntile)):
                                    w = tsz[t]
                                    off = (t - r * 4) * P
                                    ptr = ps_tr.tile([P, D], bf16, tag="tr",
                                                     space="PSUM")
                                    nc.tensor.transpose(ptr[:w, :],
                                                        ht[:, off:off + w],
                                                        ident[:, :])
                                    wr = wpool.tile([P, D], f32, tag="wr")
                                    nc.vector.tensor_copy(wr[:w, :],
                                                          ptr[:w, :])
                                    nc.sync.dma_start(
                                        out=tables[par][t * P:t * P + w, :],
                                        in_=wr[:w, :])
                                    sbc = nc.vector.tensor_copy(
                                        sendbuf[:w, t, :], ptr[:w, :])
                                    for g in dve_g:
                                        _dep(sbc, g)
                                    anch["DVE"] = sbc
                                    if t + 1 == units[ui][1]:
                                        emit_unit_sends(ui, pool_g)
                                        ui += 1
                            else:
                                nc.vector.tensor_copy(
                                    act3[:, r * RW:r * RW + rw], at[:, :rw])
                        if not last:
                            emit_drains_and_ack(pub)
                            st["pub"] = pub + 1

                # ---- pooling
                first_seen = set()
                for (t, s0, s1, g) in cfg["pool_segs"]:
                    tmp = wpool.tile([D, 1], f32, tag="ptmp")
                    nc.vector.reduce_max(tmp[:],
                                         act3[:, t * P + s0:t * P + s1],
                                         axis=mybir.AxisListType.X)
                    if g not in first_seen:
                        first_seen.add(g)
                        nc.vector.tensor_copy(emb[:, g:g + 1], tmp[:])
                    else:
                        nc.vector.tensor_tensor(out=emb[:, g:g + 1],
                                                in0=emb[:, g:g + 1],
                                                in1=tmp[:],
                                                op=mybir.AluOpType.max)

                # ---- head
                emb_bf = wpool.tile([D, gpc], bf16, tag="embbf")
                nc.vector.tensor_copy(emb_bf[:], emb[:])
                ph = ps_z.tile([D, gpc], f32, tag="zt", space="PSUM")
                nc.tensor.matmul(ph[:], lhsT=l1w_sb[:], rhs=emb_bf[:],
                                 start=True, stop=True)
                h1 = wpool.tile([D, gpc], bf16, tag="h1")
                nc.scalar.activation(h1[:], ph[:],
                                     mybir.ActivationFunctionType.Relu,
                                     bias=l1b_sb[:])
                po = ps_tr.tile([ncls, gpc], f32, tag="tr", space="PSUM")
                nc.tensor.matmul(po[:], lhsT=l2w_sb[:], rhs=h1[:],
                                 start=True, stop=True)
                osb = wpool.tile([ncls, gpc], f32, tag="osb")
                nc.scalar.activation(osb[:], po[:],
                                     mybir.ActivationFunctionType.Identity,
                                     bias=l2b_sb[:])
                nc.sync.dma_start(out=out[:, :].rearrange("g c -> c g"),
                                  in_=osb[:])

                # ---- end-of-exec sem rebase (subtract this exec's totals;
                # order-insensitive vs cross-exec in-flight arrivals)
                assert st["pub"] == n_pubs and st["rnd"] == n_rounds
                w1 = _wge(nc.gpsimd, "POOL", ack_rsem, 16 * n_pubs)
                prev = w1
                for u in range(NU):
                    wz = nc.gpsimd.wait_ge(rsem_d[u], 14 * n_pubs)
                    _dep(wz, prev)
                    prev = wz
                wz = nc.gpsimd.wait_ge(rsem_s, 14 * n_rounds)
                _dep(wz, prev)
                prev = wz
                # No rebase: engine-side writes to RDMA-fed sems crash the
                # runtime (negative sem_inc AND sem_clear both). NRT resets
                # kernel semaphores at NEFF launch, so each exec starts from
                # zero; the quiescence waits above ensure no RDMA is in
                # flight when this exec completes.

        nc.compile()
    return nc


# ---------------------------------------------------------------- entry point

_CACHE = {}


def _get_built(cfg_key, cfg, reps):
    key = (cfg_key, reps)
    if key not in _CACHE:
        _CACHE[key] = _build(cfg, reps=reps)
    return _CACHE[key]


def _in_maps(x, data, cfg, W1, W2, W3, gamma, beta,
             lin1_w, lin1_b, lin2_w, lin2_b):
    nsh, ncls = cfg["nsh"], cfg["n_classes"]
    W_bf = [np.asarray(w, np.float32).astype(ml_dtypes.bfloat16)
            for w in (W1, W2, W3)]
    maps = []
    for c in range(N_CORES):
        maps.append({
            "x_sh": x[c * nsh:(c + 1) * nsh].astype(np.float32),
            "idx": data["idx_rep"][c],
            "O": data["Ot"][c],
            "deg_pt": data["deg_pt"][c],
            "deg_row": data["deg_row"][c],
            "W1": W_bf[0], "W2": W_bf[1], "W3": W_bf[2],
            "gamma": np.asarray(gamma, np.float32).reshape(D, 1),
            "beta": np.asarray(beta, np.float32).reshape(D, 1),
            "lin1w": np.asarray(lin1_w, np.float32).astype(ml_dtypes.bfloat16),
            "lin1b": np.asarray(lin1_b, np.float32).reshape(D, 1),
            "lin2w": np.asarray(lin2_w, np.float32).astype(ml_dtypes.bfloat16),
            "lin2b": np.asarray(lin2_b, np.float32).reshape(ncls, 1),
        })
    return maps


def kernel(x, edge_index, batch, W1, b1, W2, b2, W3, b3, gamma, beta,
           lin1_w, lin1_b, lin2_w, lin2_b, _reps=1):
    x = np.asarray(x, np.float32)
    edge_index = np.asarray(edge_index)
    batch = np.asarray(batch)
    n_nodes, d = x.shape
    ncls = np.asarray(lin2_w).shape[1]
    assert d == D

    cfg, data = _prep(x, edge_index, batch, ncls)

    # NOTE: b1/b2/b3 cancel inside BatchNorm (mean subtraction) - unused.
    in_maps = _in_maps(x, data, cfg, W1, W2, W3, gamma, beta,
                       lin1_w, lin1_b, lin2_w, lin2_b)
    cfg_key = (n_nodes, edge_index.shape[1], ncls)
    nc = _get_built(cfg_key, cfg, _reps)
    res = run_bass_kernel_spmd(nc, in_maps, core_ids=list(range(N_CORES)))
    outs = [res.results[c]["out"] for c in range(N_CORES)]
    return np.concatenate(outs, axis=0).astype(np.float32)


# revision 35
# speedup vs baseline: 1.1491x; 1.1491x over previous
"""Trainium2 Bass kernel for nn_GCN1PoolNorm: 3-layer GCN + shared BatchNorm +
global max pool + MLP head, SPMD across 8 NeuronCores.

Self-contained: takes FULL inputs, returns FULL output [N_GRAPHS, N_CLASSES].

Design (per core = one 1/8 dst-shard of nodes) — SPARSE gather + one-hot matmul:
- Node table h_tilde = act * dis lives in Shared DRAM as [n_nodes, 64] fp32
  (256B rows — the dma_gather element granularity), AllGather-published per
  layer.
- Per layer, each core gathers the h_tilde rows of its incident edges' src
  nodes with dma_gather (max 1024 idx per call = SWDGE ring limit; calls
  round-robin the 4 SWDGE queues so ring drains overlap desc generation).
  int16 gather indices cap the table at 32768 rows, so edges are split into
  lo (src < 32768) and hi groups per 512-dst range, sorted by
  (dst_range, src_half, dst) so the gather stream is one monotone chunk
  sequence.
- Aggregation agg[dst] = sum_e h_tilde[src_e] runs range-major: one PSUM
  bank [64, 512] per dst range; per node tile one contiguous accumulation
  group of matmuls psum[:, tile] += M_block.T @ O_block, where M_block
  [128, 64] is the gathered (bf16-converted) 128-edge block and O_block
  [128, 128] fp8 is a host-built one-hot edge->dst_local matrix. Exact;
  PSUM accumulation handles duplicate dsts; dis[dst] factors out of the
  sum. Self-loops are extra edges (src = dst).
- Downstream per range reads the bank: U.T = psum * dis_rep; Z.T = W.T@U.T;
  BN stats via ACT accum_out; stats AllReduce; BN affine+relu fused; * dis;
  PE transpose per tile; DMA to table shard; AllGather.
- Pooling: graphs align exactly to cores; free-axis reduce_max segments;
  MLP head feat-major; out [gpc, 10] per core, host concatenates.
"""
import numpy as np
import ml_dtypes

from concourse import bacc, mybir, tile
from concourse.bass_utils import run_bass_kernel_spmd
from concourse.masks import make_identity

f32 = mybir.dt.float32
bf16 = mybir.dt.bfloat16
fp8 = mybir.dt.float8e4
i16 = mybir.dt.int16

N_CORES = 8
ABLATE = set()   # sim-only ablation knob ("coll")
P = 128          # partition / block quantum
D = 64           # feature dim
HALF = 32768     # int16 gather index limit -> lo/hi table split
CC = 16          # gather chunk columns (CC*128 edge slots per chunk)
GC = 4           # columns per dma_gather call (1024 idx = SWDGE ring limit)
OG = 32          # one-hot matrices per O stream group
BN_EPS = 1e-5


# ---------------------------------------------------------------- host prep

def _prep(x, edge_index, batch, n_classes):
    n_nodes = x.shape[0]
    n_graphs = int(batch.max()) + 1
    assert n_nodes % N_CORES == 0
    nsh = n_nodes // N_CORES                    # nodes per core
    ntile = (nsh + P - 1) // P                  # node tiles per core
    tsz = [min(P, nsh - t * P) for t in range(ntile)]

    src_all = np.asarray(edge_index[0], np.int64)
    dst_all = np.asarray(edge_index[1], np.int64)
    deg = np.bincount(dst_all, minlength=n_nodes).astype(np.int64)

    RW = 512
    RNG = (nsh + RW - 1) // RW                  # 512-node dst ranges per core

    # ---- per-core edge lists, sorted by (dst_range, src_half, dst, src):
    # each (range, half) group is a contiguous slot run so the gather
    # stream is a single monotone sequence of chunks per layer
    edges = []                                  # (s, dl, grp) per core
    for c in range(N_CORES):
        m = (dst_all // nsh) == c
        s = src_all[m]
        dl = dst_all[m] - c * nsh
        # (self loops are NOT edges here: the self contribution h_tilde[dst]
        # is added from the resident feature-major hbuf after aggregation)
        hi = (s >= HALF).astype(np.int64)
        grp = (dl // RW) * 2 + hi
        order = np.lexsort((s, dl, grp))
        s, dl, grp = s[order], dl[order], grp[order]
        edges.append((s, dl, grp))

    # SPMD = one program for all cores: pad every (range, half) group to
    # the max block count over cores; pad slots gather table row 0 of the
    # group's half and are masked by all-zero O columns. Schedules are the
    # union over cores; a core lacking a (group, tile, block) gets an
    # all-zero O matrix (adds 0 to psum).
    NGRP = RNG * 2
    gcols = np.zeros(NGRP, np.int64)            # block columns per group
    for c in range(N_CORES):
        cnt = np.bincount(edges[c][2], minlength=NGRP)
        gcols = np.maximum(gcols, (cnt + P - 1) // P)
    gbase = np.concatenate([[0], np.cumsum(gcols)])  # group -> base col
    SC = int(gbase[-1])

    keys = []
    per_edge = []
    for c in range(N_CORES):
        s, dl, grp = edges[c]
        # slot within group run
        gstart = np.searchsorted(grp, np.arange(NGRP))
        slot = gbase[grp] * P + (np.arange(s.shape[0]) - gstart[grp])
        b = slot // P
        t = dl // P
        hi = grp & 1
        # mm issue order: tile-major, each tile's lo mms then hi mms — the
        # tile's matmuls are one contiguous PSUM accumulation group; chunk
        # consumption steps back at most one range's group span
        key = ((t * 2 + hi) << 24) | b
        keys.append(key)
        per_edge.append((s, dl, slot))
    uk = np.unique(np.concatenate(keys))
    nmm = int(uk.shape[0])
    uk_t = uk >> 25
    uk_b = uk & ((1 << 24) - 1)
    # sched[t] = ordered (mi, b) list (lo blocks then hi blocks)
    sched = [[] for _ in range(ntile)]
    for mi in range(nmm):
        sched[int(uk_t[mi])].append((mi, int(uk_b[mi])))

    # gather chunks: (base_col, ncols, is_hi), never spanning a group
    chunks = []
    for g in range(NGRP):
        for c0 in range(0, int(gcols[g]), CC):
            chunks.append((int(gbase[g]) + c0,
                           int(min(CC, gcols[g] - c0)), g & 1))

    NG = (nmm + OG - 1) // OG
    idx_reps, Ots = [], []
    for c in range(N_CORES):
        s, dl, slot = per_edge[c]
        hi_e = s >= HALF
        # gather index array, 16-wrapped and replicated to 128 partitions;
        # pad slots point at table row 0 (junk, masked by zero O columns)
        idx_flat = np.zeros(SC * P, np.int16)
        idx_flat[slot[~hi_e]] = s[~hi_e].astype(np.int16)
        idx_flat[slot[hi_e]] = (s[hi_e] - HALF).astype(np.int16)
        wrap = idx_flat.reshape(SC * 8, 16).T            # [16, SC*8]
        idx_reps.append(np.tile(wrap, (8, 1)))           # [128, SC*8]

        inv = np.searchsorted(uk, keys[c])
        O = np.zeros((nmm, P, P), np.uint8)
        O[inv, slot % P, dl % P] = 1
        # O stream layout: [NG, 128, OG, 128] fp8, group g col j = O[g*OG+j]
        Ot = np.zeros((NG, P, OG, P), ml_dtypes.float8_e4m3)
        Of = O.astype(ml_dtypes.float8_e4m3)
        for g in range(NG):
            k = min(OG, nmm - g * OG)
            Ot[g, :, :k, :] = Of[g * OG:g * OG + k].transpose(1, 0, 2)
        Ots.append(Ot)
        del O, Of

    # deg layouts (fp32)
    deg_pt = np.zeros((N_CORES, P, ntile), np.float32)
    deg_row = np.zeros((N_CORES, 1, nsh), np.float32)
    for c in range(N_CORES):
        dsh = deg[c * nsh:(c + 1) * nsh].astype(np.float32)
        deg_row[c, 0, :] = dsh
        for tt in range(ntile):
            deg_pt[c, :tsz[tt], tt] = dsh[tt * P:tt * P + tsz[tt]]

    # pooling segments (identical across cores required for SPMD)
    gb = np.searchsorted(batch, np.arange(n_graphs + 1))
    gpc = n_graphs // N_CORES
    loc0 = gb[:gpc + 1].copy()
    for c in range(N_CORES):
        locc = gb[c * gpc:(c + 1) * gpc + 1] - c * nsh
        assert np.array_equal(locc, loc0), "graph pattern must match across cores"
    pool_segs = []
    for tt in range(ntile):
        a, bb = tt * P, tt * P + tsz[tt]
        for g in range(gpc):
            ss, ee = max(a, int(loc0[g])), min(bb, int(loc0[g + 1]))
            if ss < ee:
                pool_segs.append((tt, ss - a, ee - a, g))

    cfg = dict(n_nodes=n_nodes, nsh=nsh, ntile=ntile, tsz=tsz,
               pool_segs=pool_segs, gpc=gpc, n_classes=n_classes,
               n_graphs=n_graphs, SC=SC, NG=NG, chunks=chunks, sched=sched)
    data = dict(idx_rep=idx_reps, Ot=Ots, deg_pt=deg_pt, deg_row=deg_row)
    return cfg, data


# ---------------------------------------------------------------- device build

def _build(cfg, reps=1):
    nsh, ntile, tsz = cfg["nsh"], cfg["ntile"], cfg["tsz"]
    ncls, gpc = cfg["n_classes"], cfg["gpc"]
    n_nodes = cfg["n_nodes"]
    SC, NG = cfg["SC"], cfg["NG"]
    nshp = ntile * P
    RW = 512

    sched0 = cfg["sched"]

    R = (nsh + RW - 1) // RW
    rsz = [min(RW, nsh - r * RW) for r in range(R)]

    nc = bacc.Bacc(trn_type="TRN2", target_bir_lowering=False, debug=False,
                   num_devices=N_CORES, num_swdge_queues=4)

    x_sh = nc.dram_tensor("x_sh", [nsh, D], f32, kind="ExternalInput").ap()
    idx_in = nc.dram_tensor("idx", [P, SC * 8], i16, kind="ExternalInput").ap()
    O_in = nc.dram_tensor("O", [NG, P, OG, P], fp8, kind="ExternalInput").ap()
    deg_pt = nc.dram_tensor("deg_pt", [P, ntile], f32, kind="ExternalInput").ap()
    deg_row = nc.dram_tensor("deg_row", [1, nsh], f32, kind="ExternalInput").ap()
    Ws = [nc.dram_tensor(f"W{i}", [D, D], bf16, kind="ExternalInput").ap()
          for i in (1, 2, 3)]
    gamma = nc.dram_tensor("gamma", [D, 1], f32, kind="ExternalInput").ap()
    beta = nc.dram_tensor("beta", [D, 1], f32, kind="ExternalInput").ap()
    lin1w = nc.dram_tensor("lin1w", [D, D], bf16, kind="ExternalInput").ap()
    lin1b = nc.dram_tensor("lin1b", [D, 1], f32, kind="ExternalInput").ap()
    lin2w = nc.dram_tensor("lin2w", [D, ncls], bf16, kind="ExternalInput").ap()
    lin2b = nc.dram_tensor("lin2b", [ncls, 1], f32, kind="ExternalInput").ap()
    out = nc.dram_tensor("out", [gpc, ncls], f32, kind="ExternalOutput").ap()

    table = nc.dram_tensor("table", [n_nodes, D], f32, addr_space="Shared").ap()
    tshard = nc.dram_tensor("tshard", [nsh, D], f32).ap()
    stats_in = nc.dram_tensor("stats_in", [D, 2], f32).ap()
    stats_out = nc.dram_tensor("stats_out", [D, 2], f32,
                               addr_space="Shared").ap()

    with tile.TileContext(nc) as tc:
        with (
            tc.tile_pool(name="const", bufs=1) as cpool,
            tc.tile_pool(name="gath", bufs=8) as gpool,
            tc.tile_pool(name="gbf", bufs=8) as gbpool,
            tc.tile_pool(name="obuf", bufs=6) as opool,
            tc.tile_pool(name="work", bufs=3) as wpool,
            tc.tile_pool(name="psagg", bufs=2, space="PSUM") as ps_agg,
            tc.tile_pool(name="psz", bufs=2, space="PSUM") as ps_z,
            tc.tile_pool(name="pstr", bufs=2, space="PSUM") as ps_tr,
        ):
            # ---- residents
            idx_res = cpool.tile([P, SC * 8], i16)
            nc.sync.dma_start(out=idx_res[:], in_=idx_in[:])
            dis_pt = cpool.tile([P, ntile], f32)
            dis_rep = cpool.tile([D, nshp], f32)
            zbuf = cpool.tile([D, nshp], bf16)
            act3 = zbuf
            hbuf = cpool.tile([D, nshp], bf16)
            sums = cpool.tile([D, R], f32)
            sums2 = cpool.tile([D, R], f32)
            W_sb = [cpool.tile([D, D], bf16, tag=f"W{i}", name=f"W{i}_sb")
                    for i in range(3)]
            for i in range(3):
                nc.sync.dma_start(out=W_sb[i][:], in_=Ws[i][:])
            gamma_sb = cpool.tile([D, 1], f32, tag="gamma")
            beta_sb = cpool.tile([D, 1], f32, tag="beta")
            nc.sync.dma_start(out=gamma_sb[:], in_=gamma[:])
            nc.sync.dma_start(out=beta_sb[:], in_=beta[:])
            l1w_sb = cpool.tile([D, D], bf16, tag="l1w")
            l1b_sb = cpool.tile([D, 1], f32, tag="l1b")
            l2w_sb = cpool.tile([D, ncls], bf16, tag="l2w")
            l2b_sb = cpool.tile([ncls, 1], f32, tag="l2b")
            nc.sync.dma_start(out=l1w_sb[:], in_=lin1w[:])
            nc.sync.dma_start(out=l1b_sb[:], in_=lin1b[:])
            nc.sync.dma_start(out=l2w_sb[:], in_=lin2w[:])
            nc.sync.dma_start(out=l2b_sb[:], in_=lin2b[:])
            ident = cpool.tile([D, D], bf16, tag="ident")
            make_identity(nc, ident[:])
            ident128 = cpool.tile([P, P], bf16, tag="ident128")
            make_identity(nc, ident128[:])
            emb = cpool.tile([D, gpc], f32, tag="emb")
            eps_sb = cpool.tile([D, 1], f32, tag="eps")
            nc.gpsimd.memset(eps_sb[:], BN_EPS)

            # ---- dis
            dptf = wpool.tile([P, ntile], f32, tag="dptf")
            nc.sync.dma_start(out=dptf[:], in_=deg_pt[:])
            nc.scalar.activation(dis_pt[:], dptf[:],
                                 mybir.ActivationFunctionType.Sqrt, bias=1.0)
            nc.vector.reciprocal(dis_pt[:], dis_pt[:])
            ones1 = cpool.tile([1, D], bf16, tag="ones1")
            nc.gpsimd.memset(ones1[:], 1.0)
            for o in range(0, nsh, RW):
                w = min(RW, nsh - o)
                dsl = wpool.tile([1, RW], f32, tag="dsl")
                nc.sync.dma_start(out=dsl[:, :w], in_=deg_row[:, o:o + w])
                nc.scalar.activation(dsl[:, :w], dsl[:, :w],
                                     mybir.ActivationFunctionType.Sqrt, bias=1.0)
                nc.vector.reciprocal(dsl[:, :w], dsl[:, :w])
                dslb = wpool.tile([1, RW], bf16, tag="dslb")
                nc.vector.tensor_copy(dslb[:, :w], dsl[:, :w])
                pb = ps_z.tile([D, RW], f32, tag="zt", space="PSUM")
                nc.tensor.matmul(pb[:, :w], lhsT=ones1[:], rhs=dslb[:, :w],
                                 start=True, stop=True)
                nc.vector.tensor_copy(dis_rep[:, o:o + w], pb[:, :w])

            # ---- table0 = fp32(x * dis)
            for t in range(ntile):
                w = tsz[t]
                xt = wpool.tile([P, D], f32, tag="xt")
                nc.sync.dma_start(out=xt[:w, :], in_=x_sh[t * P:t * P + w, :])
                xb = wpool.tile([P, D], f32, tag="xb")
                nc.scalar.activation(xb[:w, :], xt[:w, :],
                                     mybir.ActivationFunctionType.Copy,
                                     scale=dis_pt[:w, t:t + 1])
                nc.sync.dma_start(out=tshard[t * P:t * P + w, :], in_=xb[:w, :])
                xbb = wpool.tile([P, D], bf16, tag="xbb")
                nc.vector.tensor_copy(xbb[:w, :], xb[:w, :])
                pxt = ps_tr.tile([D, P], bf16, tag="trx", space="PSUM")
                nc.tensor.transpose(pxt[:, :w], xbb[:w, :],
                                    ident128[:w, :w])
                nc.vector.tensor_copy(hbuf[:, t * P:t * P + w], pxt[:, :w])
            if "coll" not in ABLATE:
                nc.gpsimd.collective_compute(
                    "AllGather", mybir.AluOpType.bypass,
                    replica_groups=[list(range(N_CORES))],
                    ins=[tshard[:, :].opt()], outs=[table[:, :].opt()])

            # chunk lookup: block col -> chunk index
            chunks = cfg["chunks"]
            n_ch = len(chunks)
            col2chunk = {}
            for k, (base, ncols, _) in enumerate(chunks):
                for cc_ in range(base, base + ncols):
                    col2chunk[cc_] = k

            # ---- layers
            # global chunk counter: gather pool slot (k%8) must always pair
            # with the same SWDGE queue (k%4) — a DMA sem is locked to the
            # first queue that updates it
            qrr = [0]
            for rep in range(reps):
                for li in range(3):
                    last = (li == 2)
                    Wl = W_sb[li]

                    # lazily-issued gather chunks, one monotone stream
                    chunk_tiles = [None] * n_ch
                    next_issue = [0]

                    def issue_chunk(k):
                        base, ncols, ih = chunks[k]
                        g = gpool.tile([P, CC, D], f32, tag="g")
                        src_ap = table[HALF:n_nodes, :] if ih \
                            else table[0:HALF, :]
                        # <=1024 idx per call (SWDGE ring limit). Tile
                        # assigns each Pool DMA a DMASW sem lane round-robin
                        # mod 8, and a lane's sem is locked to one SWDGE
                        # queue — queue = global call counter % 4 keeps the
                        # lane->queue pairing consistent (8 % 4 == 0).
                        for q0 in range(0, ncols, GC):
                            qw = min(GC, ncols - q0)
                            nc.gpsimd.dma_gather(
                                out_ap=g[:, q0:q0 + qw, :], in_ap=src_ap,
                                idxs_ap=idx_res[:, (base + q0) * 8:
                                                (base + q0 + qw) * 8],
                                num_idxs=qw * P, num_idxs_reg=qw * P,
                                elem_size=D,
                                queue_num=qrr[0] % 4)
                            qrr[0] += 1
                        gb = gbpool.tile([P, CC, D], bf16, tag="gb")
                        nc.scalar.activation(
                            gb[:, :ncols, :], g[:, :ncols, :],
                            mybir.ActivationFunctionType.Copy)
                        chunk_tiles[k] = (gb, base, ncols)

                    def chunk_of(bcol):
                        k = col2chunk[bcol]
                        while next_issue[0] <= k:
                            issue_chunk(next_issue[0])
                            next_issue[0] += 1
                        return chunk_tiles[k]

                    # O group stream
                    o_tiles = [None] * NG

                    def o_tile(mi):
                        g = mi // OG
                        if o_tiles[g] is None:
                            ot = opool.tile([P, OG, P], fp8, tag="O")
                            nc.sync.dma_start(out=ot[:], in_=O_in[g])
                            o_tiles[g] = ot
                        return o_tiles[g]

                    # aggregation + downstream, range-major: one PSUM bank
                    # [64, 512] per range; per tile one accumulation group
                    # (its lo mms then its hi mms), downstream reads the bank
                    def downstream(r, ps):
                        rw = rsz[r]
                        us = wpool.tile([D, RW], f32, tag="us")
                        nc.vector.tensor_tensor(
                            out=us[:, :rw], in0=ps[:, :rw],
                            in1=hbuf[:, r * RW:r * RW + rw],
                            op=mybir.AluOpType.add)
                        u2t = wpool.tile([D, RW], bf16, tag="u2t")
                        nc.vector.tensor_tensor(
                            out=u2t[:, :rw], in0=us[:, :rw],
                            in1=dis_rep[:, r * RW:r * RW + rw],
                            op=mybir.AluOpType.mult)
                        psz = ps_z.tile([D, RW], f32, tag="zt", space="PSUM")
                        nc.tensor.matmul(psz[:, :rw], lhsT=Wl[:],
                                         rhs=u2t[:, :rw],
                                         start=True, stop=True)
                        zslice = (act3 if last else zbuf)[:, r * RW:r * RW + rw]
                        nc.scalar.activation(
                            zslice, psz[:, :rw],
                            mybir.ActivationFunctionType.Copy,
                            accum_out=sums[:, r:r + 1])
                        sq = wpool.tile([D, RW], f32, tag="sq")
                        nc.scalar.activation(
                            sq[:, :rw], psz[:, :rw],
                            mybir.ActivationFunctionType.Square,
                            accum_out=sums2[:, r:r + 1])

                    for r in range(R):
                        rw = rsz[r]
                        ps = ps_agg.tile([D, RW], f32, tag="agg",
                                         space="PSUM")
                        for t in range(r * 4, min(r * 4 + 4, ntile)):
                            mms = sched0[t]
                            assert mms, "tile with no edges"
                            tw = tsz[t]
                            off = (t - r * 4) * P
                            nmms = len(mms)
                            for j, (mi, b) in enumerate(mms):
                                gb, base, ncols = chunk_of(b)
                                osb = o_tile(mi)
                                nc.tensor.matmul(
                                    ps[:, off:off + tw],
                                    lhsT=gb[:, b - base, :],
                                    rhs=osb[:, mi % OG, :tw],
                                    start=(j == 0), stop=(j == nmms - 1))
                        downstream(r, ps)

                    # ---- global BN stats
                    st = wpool.tile([D, 2], f32, tag="st")
                    nc.vector.reduce_sum(st[:, 0:1], sums[:],
                                         axis=mybir.AxisListType.X)
                    nc.vector.reduce_sum(st[:, 1:2], sums2[:],
                                         axis=mybir.AxisListType.X)
                    nc.sync.dma_start(out=stats_in[:], in_=st[:])
                    if "coll" not in ABLATE:
                        nc.gpsimd.collective_compute(
                            "AllReduce", mybir.AluOpType.add,
                            replica_groups=[list(range(N_CORES))],
                            ins=[stats_in[:, :].opt()],
                            outs=[stats_out[:, :].opt()])
                    stg = wpool.tile([D, 2], f32, tag="stg")
                    nc.sync.dma_start(out=stg[:], in_=stats_out[:])
                    mu = wpool.tile([D, 1], f32, tag="mu")
                    nc.scalar.activation(mu[:], stg[:, 0:1],
                                         mybir.ActivationFunctionType.Copy,
                                         scale=1.0 / n_nodes)
                    va = wpool.tile([D, 1], f32, tag="va")
                    nc.scalar.activation(va[:], stg[:, 1:2],
                                         mybir.ActivationFunctionType.Copy,
                                         scale=1.0 / n_nodes)
                    mu2 = wpool.tile([D, 1], f32, tag="mu2")
                    nc.vector.tensor_tensor(out=mu2[:], in0=mu[:], in1=mu[:],
                                            op=mybir.AluOpType.mult)
                    nc.vector.tensor_tensor(out=va[:], in0=va[:], in1=mu2[:],
                                            op=mybir.AluOpType.subtract)
                    nc.scalar.activation(va[:], va[:],
                                         mybir.ActivationFunctionType.Sqrt,
                                         bias=eps_sb[:])
                    nc.vector.reciprocal(va[:], va[:])
                    saff = wpool.tile([D, 1], f32, tag="saff")
                    nc.vector.tensor_tensor(out=saff[:], in0=gamma_sb[:],
                                            in1=va[:], op=mybir.AluOpType.mult)
                    tsh_ = wpool.tile([D, 1], f32, tag="tsh")
                    nc.vector.tensor_tensor(out=tsh_[:], in0=mu[:], in1=saff[:],
                                            op=mybir.AluOpType.mult)
                    nc.vector.tensor_tensor(out=tsh_[:], in0=beta_sb[:],
                                            in1=tsh_[:],
                                            op=mybir.AluOpType.subtract)

                    # ---- activation phase (per range)
                    for r in range(R):
                        rw = rsz[r]
                        zsl = (act3 if last else zbuf)[:, r * RW:r * RW + rw]
                        at = wpool.tile([D, RW], bf16, tag="at")
                        nc.scalar.activation(at[:, :rw], zsl,
                                             mybir.ActivationFunctionType.Relu,
                                             bias=tsh_[:], scale=saff[:])
                        if not last:
                            ht = hbuf[:, r * RW:r * RW + rw]
                            nc.vector.tensor_tensor(
                                out=ht, in0=at[:, :rw],
                                in1=dis_rep[:, r * RW:r * RW + rw],
                                op=mybir.AluOpType.mult)
                            for t in range(r * 4, min(r * 4 + 4, ntile)):
                                w = tsz[t]
                                off = (t - r * 4) * P
                                ptr = ps_tr.tile([P, D], bf16, tag="tr",
                                                 space="PSUM")
                                nc.tensor.transpose(ptr[:w, :],
                                                    ht[:, off:off + w],
                                                    ident[:, :])
                                wr = wpool.tile([P, D], f32, tag="wr")
                                nc.vector.tensor_copy(wr[:w, :], ptr[:w, :])
                                nc.sync.dma_start(
                                    out=tshard[t * P:t * P + w, :],
                                    in_=wr[:w, :])
                        else:
                            nc.vector.tensor_copy(
                                act3[:, r * RW:r * RW + rw], at[:, :rw])
                    if not last and "coll" not in ABLATE:
                        nc.gpsimd.collective_compute(
                            "AllGather", mybir.AluOpType.bypass,
                            replica_groups=[list(range(N_CORES))],
                            ins=[tshard[:, :].opt()],
                            outs=[table[:, :].opt()])

            # ---- pooling
            first_seen = set()
            for (t, s0, s1, g) in cfg["pool_segs"]:
                tmp = wpool.tile([D, 1], f32, tag="ptmp")
                nc.vector.reduce_max(tmp[:], act3[:, t * P + s0:t * P + s1],
                                     axis=mybir.AxisListType.X)
                if g not in first_seen:
                    first_seen.add(g)
                    nc.vector.tensor_copy(emb[:, g:g + 1], tmp[:])
                else:
                    nc.vector.tensor_tensor(out=emb[:, g:g + 1],
                                            in0=emb[:, g:g + 1], in1=tmp[:],
                                            op=mybir.AluOpType.max)

            # ---- head
            emb_bf = wpool.tile([D, gpc], bf16, tag="embbf")
            nc.vector.tensor_copy(emb_bf[:], emb[:])
            ph = ps_z.tile([D, gpc], f32, tag="zt", space="PSUM")
            nc.tensor.matmul(ph[:], lhsT=l1w_sb[:], rhs=emb_bf[:],
                             start=True, stop=True)
            h1 = wpool.tile([D, gpc], bf16, tag="h1")
            nc.scalar.activation(h1[:], ph[:],
                                 mybir.ActivationFunctionType.Relu,
                                 bias=l1b_sb[:])
            po = ps_tr.tile([ncls, gpc], f32, tag="tr", space="PSUM")
            nc.tensor.matmul(po[:], lhsT=l2w_sb[:], rhs=h1[:],
                             start=True, stop=True)
            osb = wpool.tile([ncls, gpc], f32, tag="osb")
            nc.scalar.activation(osb[:], po[:],
                                 mybir.ActivationFunctionType.Identity,
                                 bias=l2b_sb[:])
            nc.sync.dma_start(out=out[:, :].rearrange("g c -> c g"), in_=osb[:])

    nc.compile()
    return nc


# ---------------------------------------------------------------- entry point

_CACHE = {}


def _get_built(cfg_key, cfg, reps):
    key = (cfg_key, reps)
    if key not in _CACHE:
        _CACHE[key] = _build(cfg, reps=reps)
    return _CACHE[key]


def _in_maps(x, data, cfg, W1, W2, W3, gamma, beta,
             lin1_w, lin1_b, lin2_w, lin2_b):
    nsh, ncls = cfg["nsh"], cfg["n_classes"]
    W_bf = [np.asarray(w, np.float32).astype(ml_dtypes.bfloat16)
            for w in (W1, W2, W3)]
    maps = []
    for c in range(N_CORES):
        maps.append({
            "x_sh": x[c * nsh:(c + 1) * nsh].astype(np.float32),
            "idx": data["idx_rep"][c],
            "O": data["Ot"][c],
            "deg_pt": data["deg_pt"][c],
            "deg_row": data["deg_row"][c],
            "W1": W_bf[0], "W2": W_bf[1], "W3": W_bf[2],
            "gamma": np.asarray(gamma, np.float32).reshape(D, 1),
            "beta": np.asarray(beta, np.float32).reshape(D, 1),
            "lin1w": np.asarray(lin1_w, np.float32).astype(ml_dtypes.bfloat16),
            "lin1b": np.asarray(lin1_b, np.float32).reshape(D, 1),
            "lin2w": np.asarray(lin2_w, np.float32).astype(ml_dtypes.bfloat16),
            "lin2b": np.asarray(lin2_b, np.float32).reshape(ncls, 1),
        })
    return maps


def kernel(x, edge_index, batch, W1, b1, W2, b2, W3, b3, gamma, beta,
           lin1_w, lin1_b, lin2_w, lin2_b, _reps=1):
    x = np.asarray(x, np.float32)
    edge_index = np.asarray(edge_index)
    batch = np.asarray(batch)
    n_nodes, d = x.shape
    ncls = np.asarray(lin2_w).shape[1]
    assert d == D

    cfg, data = _prep(x, edge_index, batch, ncls)

    # NOTE: b1/b2/b3 cancel inside BatchNorm (mean subtraction) - unused.
    in_maps = _in_maps(x, data, cfg, W1, W2, W3, gamma, beta,
                       lin1_w, lin1_b, lin2_w, lin2_b)
    cfg_key = (n_nodes, edge_index.shape[1], ncls)
    nc = _get_built(cfg_key, cfg, _reps)
    res = run_bass_kernel_spmd(nc, in_maps, core_ids=list(range(N_CORES)))
    outs = [res.results[c]["out"] for c in range(N_CORES)]
    return np.concatenate(outs, axis=0).astype(np.float32)

